# revision 28
# baseline (speedup 1.0000x reference)
"""Bass/Trainium2 kernel for causal MHA (B=1, S=4096, E=1024, H=16, D=64).

Sharding: tensor-parallel over heads across 8 NeuronCores (2 heads/core).
Each core computes q/k/v projections for its 2 heads, causal softmax
attention, and a partial out-projection (contraction over its 128 context
dims). Partials are summed on the host; the output bias is added on-device
as bout/8 per core so the host sum reproduces a single bias add.

Per-core device algorithm (head-packed layout, hd = 2 heads x 64 dims):
  A) QT/KT [128(hd), S] and V [S, hd] from xT tiles (full 128x128 matmuls)
  B) per t-superblock of 512, per head: S^T = K^T.T @ Q^T in [s, t] layout,
     exp via ACT (scale=1/8 fused), causal mask on diagonal-crossing blocks
     via a multiplicative mask, then ctx' += V'.T @ A^T where V' has an
     appended ones column, so ctx'[64] accumulates the softmax denominator.
  C) reciprocal + partition-broadcast of denominators, normalize ctxT,
     out_partial = ctxT.T @ Wout (+ bout/8), DMA out.

All matmuls use float32r (full-rate fp32 PE mode at moving-dim 512).
"""

import sys

import numpy as np

if "/opt/trn_rl_repo" not in sys.path:
    sys.path.append("/opt/trn_rl_repo")

import concourse.bass as bass
import concourse.mybir as mybir
import concourse.tile as tile
from concourse import bacc
from concourse.bass_utils import run_bass_kernel_spmd
from concourse.masks import make_identity

F32 = mybir.dt.float32
F32R = mybir.dt.float32r
AF = mybir.ActivationFunctionType

S = 4096
E = 1024
H = 16
D = 64
NCORES = 8
HPC = H // NCORES  # heads per core
HD = HPC * D  # packed head dims per core (=128)

def emit(ctx, tc, xT, wq, wk, wv, bq, bk, bv, wout, bout8, masks, rs_scratch, out, s=S):
    """Emit the per-core program. s = sequence length (parameterized so the
    simulator can run a scaled-down version)."""
    nc = tc.nc
    ec = E // 128  # contraction chunks for qkv proj
    nsb = s // 512  # t-superblocks
    nsc = s // 128  # s-chunks

    const = ctx.enter_context(tc.tile_pool(name="const", bufs=1))
    big = ctx.enter_context(tc.tile_pool(name="big", bufs=1))
    xtp = ctx.enter_context(tc.tile_pool(name="xtp", bufs=3))
    atp = ctx.enter_context(tc.tile_pool(name="atp", bufs=4))
    outp = ctx.enter_context(tc.tile_pool(name="outp", bufs=3))
    psum = ctx.enter_context(tc.tile_pool(name="psum", bufs=2, space="PSUM"))

    # ---- constants / weights in SBUF ----
    wq_sb = const.tile([128, ec, HD], F32R)  # [e_in_chunk, e_chunk, hd]
    wk_sb = const.tile([128, ec, HD], F32R)
    wv_sb = const.tile([128, ec, HD], F32R)
    for w_sb, w_dram in ((wq_sb, wq), (wk_sb, wk), (wv_sb, wv)):
        nc.sync.dma_start(
            out=w_sb, in_=w_dram.rearrange("(c p) n -> p c n", p=128)
        )
    bq_sb = const.tile([HD, 1], F32)
    bk_sb = const.tile([HD, 1], F32)
    nc.sync.dma_start(
        out=bq_sb,
        in_=bass.AP(tensor=bq.tensor, offset=bq.offset, ap=[[1, HD], [1, 1]]),
    )
    nc.sync.dma_start(
        out=bk_sb,
        in_=bass.AP(tensor=bk.tensor, offset=bk.offset, ap=[[1, HD], [1, 1]]),
    )
    bv_sb = const.tile([HD, 1], F32)
    nc.sync.dma_start(
        out=bv_sb,
        in_=bass.AP(tensor=bv.tensor, offset=bv.offset, ap=[[1, HD], [1, 1]]),
    )
    wout_sb = const.tile([HD, E], F32R)
    nc.sync.dma_start(out=wout_sb, in_=wout)
    bout_rep = const.tile([128, E], F32)
    nc.sync.dma_start(
        out=bout_rep,
        in_=bass.AP(
            tensor=bout8.tensor, offset=bout8.offset, ap=[[0, 128], [1, E]]
        ),
    )
    masks_sb = const.tile([128, 4, 512], F32)
    nc.sync.dma_start(out=masks_sb, in_=masks.rearrange("p (r n) -> p r n", r=4))
    ident32 = const.tile([128, 128], F32)
    make_identity(nc, ident32)
    ident = const.tile([128, 128], F32R)
    nc.vector.tensor_copy(ident, ident32)
    ones32 = const.tile([128, 1], F32)
    nc.vector.memset(ones32, 1.0)

    # ---- persistent activations ----
    qt_sb = big.tile([128, s], F32R)  # [hd, t]
    kt_sb = big.tile([128, s], F32R)  # [hd, s]
    v_sb = big.tile([128, nsc, HPC, D + 1], F32R)  # [s_in_chunk, chunk, h, d']
    ctxT_sb = big.tile([128, s], F32R)  # [hd, t]
    rowsum = big.tile([1, HPC, s], F32)
    rinv_rep = big.tile([128, s], F32)

    # V' ones column (free-dim-broadcast copy so the f32r write is rounded)
    nc.vector.tensor_copy(
        v_sb[:, :, :, D : D + 1],
        bass.AP(
            tensor=ones32.tensor,
            offset=ones32.offset,
            ap=[list(ones32.ap[0]), [0, nsc], [0, HPC], [0, 1]],
        ),
    )

    # ---- phase A: qkv projection ----
    # Manual double-buffer: q/k accumulators use the two banks of one
    # 2-bank slot (tags st0/st1 are reused by phase B's score pairs).
    ps_qq = psum.tile([128, 2, 512], F32, tag="st0", bufs=1)
    ps_kk = psum.tile([128, 2, 512], F32, tag="st1", bufs=1)
    vts = []

    def emit_transposes(sb, vt):
        for r in range(4):
            j = sb * 4 + r
            ps_t = psum.tile([128, 128], F32R, tag="vv", name="ps_t")
            nc.tensor.transpose(ps_t, vt[:, r * 128 : (r + 1) * 128], ident)
            for h in range(HPC):
                nc.vector.tensor_copy(
                    v_sb[:, j, h, 0:D], ps_t[:, h * D : (h + 1) * D]
                )

    for sb in range(nsb):
        tsl = bass.ts(sb, 512)
        ps_q = ps_qq[:, sb % 2, :]
        ps_k = ps_kk[:, sb % 2, :]
        ps_v = psum.tile([128, 512], F32, tag="vv", name="ps_v")
        for e in range(ec):
            xt = xtp.tile([128, 512], F32R, name="xt")
            nc.sync.dma_start(out=xt, in_=xT[e * 128 : (e + 1) * 128, tsl])
            for ps, w_sb in ((ps_q, wq_sb), (ps_k, wk_sb), (ps_v, wv_sb)):
                nc.tensor.matmul(
                    ps, w_sb[:, e, :], xt,
                    start=(e == 0), stop=(e == ec - 1),
                )
        nc.scalar.activation(qt_sb[:, tsl], ps_q, AF.Identity, bias=bq_sb)
        nc.scalar.activation(kt_sb[:, tsl], ps_k, AF.Identity, bias=bk_sb)
        vt = xtp.tile([128, 512], F32R, name="vt", tag="vt", bufs=2)
        nc.scalar.activation(vt, ps_v, AF.Identity, bias=bv_sb)
        vts.append(vt)
        # transposes for the PREVIOUS superblock (vt now drained from ACT),
        # so the PE never waits on this superblock's ACT copy
        if sb > 0:
            emit_transposes(sb - 1, vts[sb - 1])
    emit_transposes(nsb - 1, vts[nsb - 1])

    # ---- phase B: attention (+ pipelined per-superblock epilogue) ----
    # Per j-pair: score matmuls emitted h-interleaved so the two heads'
    # disjoint PE row groups (0-63 / 64-127) execute concurrently; ACT runs
    # a 1024-wide exp over the 2-bank psum pair; AV matmuls consume it one
    # pair behind. The previous superblock's normalize+out-proj is emitted
    # after this superblock's first pair so its PE work hides DVE latency.
    def emit_epilogue(sb):
        tsl = bass.ts(sb, 512)
        nc.sync.dma_start(out=rs_scratch[:, tsl], in_=rowsum[0:1, :, tsl])
        for h in range(HPC):
            nc.sync.dma_start(
                out=rinv_rep[h * D : (h + 1) * D, tsl],
                in_=bass.AP(
                    tensor=rs_scratch.tensor,
                    offset=rs_scratch.offset + h * s + sb * 512,
                    ap=[[0, D], [1, 512]],
                ),
            )
        nc.vector.reciprocal(rinv_rep[:, tsl], rinv_rep[:, tsl])
        nc.vector.tensor_mul(ctxT_sb[:, tsl], ctxT_sb[:, tsl], rinv_rep[:, tsl])
        for c in range(4 * sb, 4 * sb + 4):
            for half in range(2):
                out_ps = psum.tile([128, 512], F32, tag="vv", name="out_ps")
                nc.tensor.matmul(
                    out_ps,
                    ctxT_sb[:, c * 128 : (c + 1) * 128],
                    wout_sb[:, half * 512 : (half + 1) * 512],
                    start=True, stop=True,
                )
                out_sb = outp.tile([128, 512], F32, name="out_sb")
                nc.vector.tensor_add(
                    out_sb, out_ps, bout_rep[:, half * 512 : (half + 1) * 512]
                )
                nc.sync.dma_start(
                    out=out[
                        c * 128 : (c + 1) * 128, half * 512 : (half + 1) * 512
                    ],
                    in_=out_sb,
                )

    for sb in range(nsb):
        tsl = bass.ts(sb, 512)
        ctx_ps = {}
        for h in range(HPC):
            ctx_ps[h] = psum.tile(
                [128, 512], F32, tag=f"ctx{h}", bufs=1, name="ctx_ps"
            )
        npair = 2 * sb + 2
        for m in range(npair):
            stp = {}
            ats = {}
            for h in range(HPC):
                stp[h] = psum.tile(
                    [128, 2, 512], F32, tag=f"st{h}", bufs=1, name="stp"
                )
            for p in range(2):
                for h in range(HPC):
                    hsl = slice(h * D, (h + 1) * D)
                    nc.tensor.matmul(
                        stp[h][:, p, :],
                        kt_sb[hsl, (2 * m + p) * 128 : (2 * m + p + 1) * 128],
                        qt_sb[hsl, tsl],
                        start=True, stop=True,
                    )
            for h in range(HPC):
                at = atp.tile([128, 2, 512], F32R, name="at")
                ats[h] = at
                nc.scalar.activation(at, stp[h], AF.Exp, scale=0.125)
                for p in range(2):
                    r = 2 * m + p - 4 * sb
                    if r >= 0:
                        nc.vector.tensor_mul(
                            at[:, p, :], at[:, p, :], masks_sb[:, r, :]
                        )
            for h in range(HPC):
                for p in range(2):
                    j = 2 * m + p
                    nc.tensor.matmul(
                        ctx_ps[h][0 : D + 1, :],
                        v_sb[:, j, h, :],
                        ats[h][:, p, :],
                        start=(j == 0), stop=(j == 2 * npair - 1),
                    )
            if sb > 0 and m == min(2, npair - 1):
                emit_epilogue(sb - 1)
        for h in range(HPC):
            hsl = slice(h * D, (h + 1) * D)
            nc.scalar.copy(ctxT_sb[hsl, tsl], ctx_ps[h][0:D, :])
            nc.vector.tensor_copy(rowsum[0:1, h, tsl], ctx_ps[h][D : D + 1, :])
    emit_epilogue(nsb - 1)


def build(s=S):
    nc = bacc.Bacc(
        "TRN2",
        target_bir_lowering=False,
        debug=False,
        num_devices=NCORES,
    )
    xT = nc.dram_tensor("xT", [E, s], F32R, kind="ExternalInput")
    wq = nc.dram_tensor("wq", [E, HD], F32R, kind="ExternalInput")
    wk = nc.dram_tensor("wk", [E, HD], F32R, kind="ExternalInput")
    wv = nc.dram_tensor("wv", [E, HD], F32R, kind="ExternalInput")
    bq = nc.dram_tensor("bq", [HD], F32, kind="ExternalInput")
    bk = nc.dram_tensor("bk", [HD], F32, kind="ExternalInput")
    bv = nc.dram_tensor("bv", [HD], F32, kind="ExternalInput")
    wout = nc.dram_tensor("wout", [HD, E], F32R, kind="ExternalInput")
    bout8 = nc.dram_tensor("bout8", [E], F32, kind="ExternalInput")
    masks = nc.dram_tensor("masks", [128, 4 * 512], F32, kind="ExternalInput")
    rs_scratch = nc.dram_tensor("rs_scratch", [HPC, s], F32)
    out = nc.dram_tensor("out", [s, E], F32, kind="ExternalOutput")

    from contextlib import ExitStack

    with tile.TileContext(nc) as tc, ExitStack() as ctx:
        emit(
            ctx, tc,
            xT[:], wq[:], wk[:], wv[:], bq[:], bk[:], bv[:],
            wout[:], bout8[:], masks[:], rs_scratch[:], out[:],
            s=s,
        )
    nc.compile()
    return nc


def make_masks():
    sl, tl = np.mgrid[0:128, 0:512]
    m = np.zeros((128, 4, 512), np.float32)
    for r in range(4):
        m[:, r, :] = (tl - 128 * r - sl >= 0).astype(np.float32)
    return np.ascontiguousarray(m.reshape(128, 4 * 512))


def make_in_maps(x, Wqkv, bqkv, Wout, bout, s=S):
    xT = np.ascontiguousarray(np.asarray(x, np.float32)[0].T)
    Wqkv = np.asarray(Wqkv, np.float32)
    bqkv = np.asarray(bqkv, np.float32)
    Wout = np.asarray(Wout, np.float32)
    bout = np.asarray(bout, np.float32)
    masks = make_masks()
    bout8 = np.ascontiguousarray(bout / NCORES)

    in_maps = []
    for m in range(NCORES):
        heads = range(HPC * m, HPC * (m + 1))
        cols = np.concatenate(
            [np.arange(h * D, (h + 1) * D) for h in heads]
        )
        in_maps.append(
            {
                "xT": xT,
                "wq": np.ascontiguousarray(Wqkv[:, 0 * E + cols]),
                "wk": np.ascontiguousarray(Wqkv[:, 1 * E + cols]),
                "wv": np.ascontiguousarray(Wqkv[:, 2 * E + cols]),
                "bq": np.ascontiguousarray(bqkv[0 * E + cols]),
                "bk": np.ascontiguousarray(bqkv[1 * E + cols]),
                "bv": np.ascontiguousarray(bqkv[2 * E + cols]),
                "wout": np.ascontiguousarray(Wout[HD * m : HD * (m + 1), :]),
                "bout8": bout8,
                "masks": masks,
            }
        )
    return in_maps


_NC_CACHE = {}


def _get_nc(s=S):
    if s not in _NC_CACHE:
        _NC_CACHE[s] = build(s)
    return _NC_CACHE[s]


def kernel(x, Wqkv, bqkv, Wout, bout):
    nc = _get_nc()
    in_maps = make_in_maps(x, Wqkv, bqkv, Wout, bout)
    res = run_bass_kernel_spmd(nc, in_maps, list(range(NCORES)))
    acc = np.zeros((S, E), np.float64)
    for r in res.results:
        acc += r["out"]
    return acc.astype(np.float32).reshape(1, S, E)


# revision 29
# speedup vs baseline: 1.1904x; 1.1904x over previous
"""Bass/Trainium2 kernel for causal MHA (B=1, S=4096, E=1024, H=16, D=64).

Sharding: tensor-parallel over heads across 8 NeuronCores (2 heads/core).
Each core computes q/k/v projections for its 2 heads, causal softmax
attention, and a partial out-projection (contraction over its 128 context
dims). Partials are summed on the host; the output bias is added on-device
as bout/8 per core so the host sum reproduces a single bias add.

Per-core device algorithm (head-packed layout, hd = 2 heads x 64 dims):
  A) QT/KT [128(hd), S] and V [S, hd] from xT tiles (full 128x128 matmuls)
  B) per t-superblock of 512, per head: S^T = K^T.T @ Q^T in [s, t] layout,
     exp via ACT (scale=1/8 fused), causal mask on diagonal-crossing blocks
     via a multiplicative mask, then ctx' += V'.T @ A^T where V' has an
     appended ones column, so ctx'[64] accumulates the softmax denominator.
  C) reciprocal + partition-broadcast of denominators, normalize ctxT,
     out_partial = ctxT.T @ Wout (+ bout/8), DMA out.

All matmuls use float32r (full-rate fp32 PE mode at moving-dim 512).
"""

import sys

import numpy as np

if "/opt/trn_rl_repo" not in sys.path:
    sys.path.append("/opt/trn_rl_repo")

import concourse.bass as bass
import concourse.mybir as mybir
import concourse.tile as tile
from concourse import bacc
from concourse.bass_utils import run_bass_kernel_spmd
from concourse.masks import make_identity

F32 = mybir.dt.float32
F32R = mybir.dt.float32r
BF16 = mybir.dt.bfloat16
AF = mybir.ActivationFunctionType

# bf16 operands for the attention matmuls (q/k/A/V): PE runs bf16 at
# 1 cycle/row vs fp32r's 2, and attention dominates PE time. qkv- and
# out-projections stay f32r; accumulation is always fp32 in PSUM.
ATT_BF16 = True
ATT_DT = BF16 if ATT_BF16 else F32R

S = 4096
E = 1024
H = 16
D = 64
NCORES = 8
HPC = H // NCORES  # heads per core
HD = HPC * D  # packed head dims per core (=128)

def emit(ctx, tc, xT, wq, wk, wv, bq, bk, bv, wout, bout8, masks, rs_scratch, out, s=S):
    """Emit the per-core program. s = sequence length (parameterized so the
    simulator can run a scaled-down version)."""
    nc = tc.nc
    ec = E // 128  # contraction chunks for qkv proj
    nsb = s // 512  # t-superblocks
    nsc = s // 128  # s-chunks

    const = ctx.enter_context(tc.tile_pool(name="const", bufs=1))
    big = ctx.enter_context(tc.tile_pool(name="big", bufs=1))
    xtp = ctx.enter_context(tc.tile_pool(name="xtp", bufs=3))
    atp = ctx.enter_context(tc.tile_pool(name="atp", bufs=4))
    outp = ctx.enter_context(tc.tile_pool(name="outp", bufs=3))
    psum = ctx.enter_context(tc.tile_pool(name="psum", bufs=2, space="PSUM"))

    # ---- constants / weights in SBUF ----
    wq_sb = const.tile([128, ec, HD], F32R)  # [e_in_chunk, e_chunk, hd]
    wk_sb = const.tile([128, ec, HD], F32R)
    wv_sb = const.tile([128, ec, HD], F32R)
    for w_sb, w_dram in ((wq_sb, wq), (wk_sb, wk), (wv_sb, wv)):
        nc.sync.dma_start(
            out=w_sb, in_=w_dram.rearrange("(c p) n -> p c n", p=128)
        )
    bq_sb = const.tile([HD, 1], F32)
    bk_sb = const.tile([HD, 1], F32)
    nc.sync.dma_start(
        out=bq_sb,
        in_=bass.AP(tensor=bq.tensor, offset=bq.offset, ap=[[1, HD], [1, 1]]),
    )
    nc.sync.dma_start(
        out=bk_sb,
        in_=bass.AP(tensor=bk.tensor, offset=bk.offset, ap=[[1, HD], [1, 1]]),
    )
    bv_sb = const.tile([HD, 1], F32)
    nc.sync.dma_start(
        out=bv_sb,
        in_=bass.AP(tensor=bv.tensor, offset=bv.offset, ap=[[1, HD], [1, 1]]),
    )
    wout_sb = const.tile([HD, E], F32R)
    nc.sync.dma_start(out=wout_sb, in_=wout)
    bout_rep = const.tile([128, E], F32)
    nc.sync.dma_start(
        out=bout_rep,
        in_=bass.AP(
            tensor=bout8.tensor, offset=bout8.offset, ap=[[0, 128], [1, E]]
        ),
    )
    masks_sb = const.tile([128, 4, 512], F32)
    nc.sync.dma_start(out=masks_sb, in_=masks.rearrange("p (r n) -> p r n", r=4))
    ident32 = const.tile([128, 128], F32)
    make_identity(nc, ident32)
    ident = const.tile([128, 128], F32R)
    nc.vector.tensor_copy(ident, ident32)
    ones32 = const.tile([128, 1], F32)
    nc.vector.memset(ones32, 1.0)

    # ---- persistent activations ----
    qt_sb = big.tile([128, s], ATT_DT)  # [hd, t]
    kt_sb = big.tile([128, s], ATT_DT)  # [hd, s]
    v_sb = big.tile([128, nsc, HPC, D + 1], ATT_DT)  # [s_in_chunk, chunk, h, d']
    ctxT_sb = big.tile([128, s], F32R)  # [hd, t]
    rowsum = big.tile([1, HPC, s], F32)
    rinv_rep = big.tile([128, s], F32)

    # V' ones column (free-dim-broadcast copy so the f32r write is rounded)
    nc.vector.tensor_copy(
        v_sb[:, :, :, D : D + 1],
        bass.AP(
            tensor=ones32.tensor,
            offset=ones32.offset,
            ap=[list(ones32.ap[0]), [0, nsc], [0, HPC], [0, 1]],
        ),
    )

    # ---- phase A: qkv projection ----
    # Manual double-buffer: q/k accumulators use the two banks of one
    # 2-bank slot (tags st0/st1 are reused by phase B's score pairs).
    ps_qq = psum.tile([128, 2, 512], F32, tag="st0", bufs=1)
    ps_kk = psum.tile([128, 2, 512], F32, tag="st1", bufs=1)
    vts = []

    def emit_transposes(sb, vt):
        for r in range(4):
            j = sb * 4 + r
            ps_t = psum.tile([128, 128], F32R, tag="vv", name="ps_t")
            nc.tensor.transpose(ps_t, vt[:, r * 128 : (r + 1) * 128], ident)
            for h in range(HPC):
                nc.vector.tensor_copy(
                    v_sb[:, j, h, 0:D], ps_t[:, h * D : (h + 1) * D]
                )

    for sb in range(nsb):
        tsl = bass.ts(sb, 512)
        ps_q = ps_qq[:, sb % 2, :]
        ps_k = ps_kk[:, sb % 2, :]
        ps_v = psum.tile([128, 512], F32, tag="vv", name="ps_v")
        for e in range(ec):
            xt = xtp.tile([128, 512], F32R, name="xt")
            nc.sync.dma_start(out=xt, in_=xT[e * 128 : (e + 1) * 128, tsl])
            for ps, w_sb in ((ps_q, wq_sb), (ps_k, wk_sb), (ps_v, wv_sb)):
                nc.tensor.matmul(
                    ps, w_sb[:, e, :], xt,
                    start=(e == 0), stop=(e == ec - 1),
                )
        nc.scalar.activation(qt_sb[:, tsl], ps_q, AF.Identity, bias=bq_sb)
        nc.scalar.activation(kt_sb[:, tsl], ps_k, AF.Identity, bias=bk_sb)
        vt = xtp.tile([128, 512], F32R, name="vt", tag="vt", bufs=2)
        nc.scalar.activation(vt, ps_v, AF.Identity, bias=bv_sb)
        vts.append(vt)
        # transposes for the PREVIOUS superblock (vt now drained from ACT),
        # so the PE never waits on this superblock's ACT copy
        if sb > 0:
            emit_transposes(sb - 1, vts[sb - 1])
    emit_transposes(nsb - 1, vts[nsb - 1])

    # ---- phase B: attention (+ pipelined per-superblock epilogue) ----
    # Per j-pair: score matmuls emitted h-interleaved so the two heads'
    # disjoint PE row groups (0-63 / 64-127) execute concurrently; ACT runs
    # a 1024-wide exp over the 2-bank psum pair; AV matmuls consume it one
    # pair behind. The previous superblock's normalize+out-proj is emitted
    # after this superblock's first pair so its PE work hides DVE latency.
    def emit_epilogue(sb):
        tsl = bass.ts(sb, 512)
        nc.sync.dma_start(out=rs_scratch[:, tsl], in_=rowsum[0:1, :, tsl])
        for h in range(HPC):
            nc.sync.dma_start(
                out=rinv_rep[h * D : (h + 1) * D, tsl],
                in_=bass.AP(
                    tensor=rs_scratch.tensor,
                    offset=rs_scratch.offset + h * s + sb * 512,
                    ap=[[0, D], [1, 512]],
                ),
            )
        nc.vector.reciprocal(rinv_rep[:, tsl], rinv_rep[:, tsl])
        nc.vector.tensor_mul(ctxT_sb[:, tsl], ctxT_sb[:, tsl], rinv_rep[:, tsl])
        for c in range(4 * sb, 4 * sb + 4):
            for half in range(2):
                out_ps = psum.tile([128, 512], F32, tag="vv", name="out_ps")
                nc.tensor.matmul(
                    out_ps,
                    ctxT_sb[:, c * 128 : (c + 1) * 128],
                    wout_sb[:, half * 512 : (half + 1) * 512],
                    start=True, stop=True,
                )
                out_sb = outp.tile([128, 512], F32, name="out_sb")
                nc.vector.tensor_add(
                    out_sb, out_ps, bout_rep[:, half * 512 : (half + 1) * 512]
                )
                nc.sync.dma_start(
                    out=out[
                        c * 128 : (c + 1) * 128, half * 512 : (half + 1) * 512
                    ],
                    in_=out_sb,
                )

    for sb in range(nsb):
        tsl = bass.ts(sb, 512)
        ctx_ps = {}
        for h in range(HPC):
            ctx_ps[h] = psum.tile(
                [128, 512], F32, tag=f"ctx{h}", bufs=1, name="ctx_ps"
            )
        npair = 2 * sb + 2
        for m in range(npair):
            stp = {}
            ats = {}
            for h in range(HPC):
                stp[h] = psum.tile(
                    [128, 2, 512], F32, tag=f"st{h}", bufs=1, name="stp"
                )
            for p in range(2):
                for h in range(HPC):
                    hsl = slice(h * D, (h + 1) * D)
                    nc.tensor.matmul(
                        stp[h][:, p, :],
                        kt_sb[hsl, (2 * m + p) * 128 : (2 * m + p + 1) * 128],
                        qt_sb[hsl, tsl],
                        start=True, stop=True,
                    )
            for h in range(HPC):
                at = atp.tile([128, 2, 512], ATT_DT, name="at")
                ats[h] = at
                nc.scalar.activation(at, stp[h], AF.Exp, scale=0.125)
                for p in range(2):
                    r = 2 * m + p - 4 * sb
                    if r >= 0:
                        nc.vector.tensor_mul(
                            at[:, p, :], at[:, p, :], masks_sb[:, r, :]
                        )
            for h in range(HPC):
                for p in range(2):
                    j = 2 * m + p
                    nc.tensor.matmul(
                        ctx_ps[h][0 : D + 1, :],
                        v_sb[:, j, h, :],
                        ats[h][:, p, :],
                        start=(j == 0), stop=(j == 2 * npair - 1),
                    )
            if sb > 0 and m == min(2, npair - 1):
                emit_epilogue(sb - 1)
        for h in range(HPC):
            hsl = slice(h * D, (h + 1) * D)
            nc.scalar.copy(ctxT_sb[hsl, tsl], ctx_ps[h][0:D, :])
            nc.vector.tensor_copy(rowsum[0:1, h, tsl], ctx_ps[h][D : D + 1, :])
    emit_epilogue(nsb - 1)


def build(s=S):
    nc = bacc.Bacc(
        "TRN2",
        target_bir_lowering=False,
        debug=False,
        num_devices=NCORES,
    )
    xT = nc.dram_tensor("xT", [E, s], F32R, kind="ExternalInput")
    wq = nc.dram_tensor("wq", [E, HD], F32R, kind="ExternalInput")
    wk = nc.dram_tensor("wk", [E, HD], F32R, kind="ExternalInput")
    wv = nc.dram_tensor("wv", [E, HD], F32R, kind="ExternalInput")
    bq = nc.dram_tensor("bq", [HD], F32, kind="ExternalInput")
    bk = nc.dram_tensor("bk", [HD], F32, kind="ExternalInput")
    bv = nc.dram_tensor("bv", [HD], F32, kind="ExternalInput")
    wout = nc.dram_tensor("wout", [HD, E], F32R, kind="ExternalInput")
    bout8 = nc.dram_tensor("bout8", [E], F32, kind="ExternalInput")
    masks = nc.dram_tensor("masks", [128, 4 * 512], F32, kind="ExternalInput")
    rs_scratch = nc.dram_tensor("rs_scratch", [HPC, s], F32)
    out = nc.dram_tensor("out", [s, E], F32, kind="ExternalOutput")

    from contextlib import ExitStack

    with tile.TileContext(nc) as tc, ExitStack() as ctx:
        emit(
            ctx, tc,
            xT[:], wq[:], wk[:], wv[:], bq[:], bk[:], bv[:],
            wout[:], bout8[:], masks[:], rs_scratch[:], out[:],
            s=s,
        )
    nc.compile()
    return nc


def make_masks():
    sl, tl = np.mgrid[0:128, 0:512]
    m = np.zeros((128, 4, 512), np.float32)
    for r in range(4):
        m[:, r, :] = (tl - 128 * r - sl >= 0).astype(np.float32)
    return np.ascontiguousarray(m.reshape(128, 4 * 512))


def make_in_maps(x, Wqkv, bqkv, Wout, bout, s=S):
    xT = np.ascontiguousarray(np.asarray(x, np.float32)[0].T)
    Wqkv = np.asarray(Wqkv, np.float32)
    bqkv = np.asarray(bqkv, np.float32)
    Wout = np.asarray(Wout, np.float32)
    bout = np.asarray(bout, np.float32)
    masks = make_masks()
    bout8 = np.ascontiguousarray(bout / NCORES)

    in_maps = []
    for m in range(NCORES):
        heads = range(HPC * m, HPC * (m + 1))
        cols = np.concatenate(
            [np.arange(h * D, (h + 1) * D) for h in heads]
        )
        in_maps.append(
            {
                "xT": xT,
                "wq": np.ascontiguousarray(Wqkv[:, 0 * E + cols]),
                "wk": np.ascontiguousarray(Wqkv[:, 1 * E + cols]),
                "wv": np.ascontiguousarray(Wqkv[:, 2 * E + cols]),
                "bq": np.ascontiguousarray(bqkv[0 * E + cols]),
                "bk": np.ascontiguousarray(bqkv[1 * E + cols]),
                "bv": np.ascontiguousarray(bqkv[2 * E + cols]),
                "wout": np.ascontiguousarray(Wout[HD * m : HD * (m + 1), :]),
                "bout8": bout8,
                "masks": masks,
            }
        )
    return in_maps


_NC_CACHE = {}


def _get_nc(s=S):
    if s not in _NC_CACHE:
        _NC_CACHE[s] = build(s)
    return _NC_CACHE[s]


def kernel(x, Wqkv, bqkv, Wout, bout):
    nc = _get_nc()
    in_maps = make_in_maps(x, Wqkv, bqkv, Wout, bout)
    res = run_bass_kernel_spmd(nc, in_maps, list(range(NCORES)))
    acc = np.zeros((S, E), np.float64)
    for r in res.results:
        acc += r["out"]
    return acc.astype(np.float32).reshape(1, S, E)


# revision 30
# speedup vs baseline: 1.2734x; 1.0697x over previous
"""Bass/Trainium2 kernel for causal MHA (B=1, S=4096, E=1024, H=16, D=64).

Sharding: tensor-parallel over heads across 8 NeuronCores (2 heads/core).
Each core computes q/k/v projections for its 2 heads, causal softmax
attention, and a partial out-projection (contraction over its 128 context
dims). Partials are summed on the host; the output bias is added on-device
as bout/8 per core so the host sum reproduces a single bias add.

Per-core device algorithm (head-packed layout, hd = 2 heads x 64 dims):
  A) QT/KT [128(hd), S] and V [S, hd] from xT tiles (full 128x128 matmuls)
  B) per t-superblock of 512, per head: S^T = K^T.T @ Q^T in [s, t] layout,
     exp via ACT (scale=1/8 fused), causal mask on diagonal-crossing blocks
     via a multiplicative mask, then ctx' += V'.T @ A^T where V' has an
     appended ones column, so ctx'[64] accumulates the softmax denominator.
  C) reciprocal + partition-broadcast of denominators, normalize ctxT,
     out_partial = ctxT.T @ Wout (+ bout/8), DMA out.

All matmuls use float32r (full-rate fp32 PE mode at moving-dim 512).
"""

import sys

import numpy as np

if "/opt/trn_rl_repo" not in sys.path:
    sys.path.append("/opt/trn_rl_repo")

import concourse.bass as bass
import concourse.mybir as mybir
import concourse.tile as tile
from concourse import bacc
from concourse.bass_utils import run_bass_kernel_spmd
from concourse.masks import make_identity

F32 = mybir.dt.float32
F32R = mybir.dt.float32r
BF16 = mybir.dt.bfloat16
AF = mybir.ActivationFunctionType

# bf16 operands for the attention matmuls (q/k/A/V): PE runs bf16 at
# 1 cycle/row vs fp32r's 2, and attention dominates PE time. qkv- and
# out-projections stay f32r; accumulation is always fp32 in PSUM.
ATT_BF16 = True
ATT_DT = BF16 if ATT_BF16 else F32R

S = 4096
E = 1024
H = 16
D = 64
NCORES = 8
HPC = H // NCORES  # heads per core
HD = HPC * D  # packed head dims per core (=128)

def emit(ctx, tc, xT, wq, wk, wv, bq, bk, bv, wout, bout8, masks, rs_scratch, out, s=S):
    """Emit the per-core program. s = sequence length (parameterized so the
    simulator can run a scaled-down version)."""
    nc = tc.nc
    ec = E // 128  # contraction chunks for qkv proj
    nsb = s // 512  # t-superblocks
    nsc = s // 128  # s-chunks

    const = ctx.enter_context(tc.tile_pool(name="const", bufs=1))
    big = ctx.enter_context(tc.tile_pool(name="big", bufs=1))
    xtp = ctx.enter_context(tc.tile_pool(name="xtp", bufs=12))
    atp = ctx.enter_context(tc.tile_pool(name="atp", bufs=6))
    outp = ctx.enter_context(tc.tile_pool(name="outp", bufs=3))
    psum = ctx.enter_context(tc.tile_pool(name="psum", bufs=2, space="PSUM"))

    # ---- constants / weights in SBUF ----
    wq_sb = const.tile([128, ec, HD], F32R)  # [e_in_chunk, e_chunk, hd]
    wk_sb = const.tile([128, ec, HD], F32R)
    wv_sb = const.tile([128, ec, HD], F32R)
    for w_sb, w_dram in ((wq_sb, wq), (wk_sb, wk), (wv_sb, wv)):
        nc.sync.dma_start(
            out=w_sb, in_=w_dram.rearrange("(c p) n -> p c n", p=128)
        )
    bq_sb = const.tile([HD, 1], F32)
    bk_sb = const.tile([HD, 1], F32)
    nc.sync.dma_start(
        out=bq_sb,
        in_=bass.AP(tensor=bq.tensor, offset=bq.offset, ap=[[1, HD], [1, 1]]),
    )
    nc.sync.dma_start(
        out=bk_sb,
        in_=bass.AP(tensor=bk.tensor, offset=bk.offset, ap=[[1, HD], [1, 1]]),
    )
    bv_sb = const.tile([HD, 1], F32)
    nc.sync.dma_start(
        out=bv_sb,
        in_=bass.AP(tensor=bv.tensor, offset=bv.offset, ap=[[1, HD], [1, 1]]),
    )
    wout_sb = const.tile([HD, E], F32R)
    nc.sync.dma_start(out=wout_sb, in_=wout)
    bout_rep = const.tile([128, E], F32)
    nc.sync.dma_start(
        out=bout_rep,
        in_=bass.AP(
            tensor=bout8.tensor, offset=bout8.offset, ap=[[0, 128], [1, E]]
        ),
    )
    masks_sb = const.tile([128, 4, 512], F32)
    nc.sync.dma_start(out=masks_sb, in_=masks.rearrange("p (r n) -> p r n", r=4))
    ident32 = const.tile([128, 128], F32)
    make_identity(nc, ident32)
    ident = const.tile([128, 128], F32R)
    nc.vector.tensor_copy(ident, ident32)
    ones32 = const.tile([128, 1], F32)
    nc.vector.memset(ones32, 1.0)

    # ---- persistent activations ----
    qt_sb = big.tile([128, s], ATT_DT)  # [hd, t]
    kt_sb = big.tile([128, s], ATT_DT)  # [hd, s]
    v_sb = big.tile([128, nsc, HPC, D + 1], ATT_DT)  # [s_in_chunk, chunk, h, d']
    ctxT_sb = big.tile([128, s], F32R)  # [hd, t]

    # V' ones column (free-dim-broadcast copy so the f32r write is rounded)
    nc.vector.tensor_copy(
        v_sb[:, :, :, D : D + 1],
        bass.AP(
            tensor=ones32.tensor,
            offset=ones32.offset,
            ap=[list(ones32.ap[0]), [0, nsc], [0, HPC], [0, 1]],
        ),
    )

    # ---- phases A+B interleaved ----
    # Phase A superblocks (qkv projection) are emitted two superblocks ahead
    # of phase B (attention), injected into B's pair loop, so the PE always
    # has projection matmuls to run while ACT works through exp calls.
    # A accumulates q/k/v sequentially into single "vv"-tag psum slots so the
    # st0/st1 tags stay free for B's score pairs.
    rowsums = {}

    def emit_transposes(sb, vt):
        for r in range(4):
            j = sb * 4 + r
            ps_t = psum.tile([128, 128], F32R, tag="vv", name="ps_t")
            nc.tensor.transpose(ps_t, vt[:, r * 128 : (r + 1) * 128], ident)
            for h in range(HPC):
                nc.vector.tensor_copy(
                    v_sb[:, j, h, 0:D], ps_t[:, h * D : (h + 1) * D]
                )

    vts = {}

    def emit_a(sb):
        tsl = bass.ts(sb, 512)
        xts = []
        for e in range(ec):
            xt = xtp.tile([128, 512], F32R, name="xt")
            nc.sync.dma_start(out=xt, in_=xT[e * 128 : (e + 1) * 128, tsl])
            xts.append(xt)
        for w_sb, b_sb, dest in (
            (wq_sb, bq_sb, qt_sb), (wk_sb, bk_sb, kt_sb), (wv_sb, bv_sb, None)
        ):
            ps = psum.tile([128, 512], F32, tag="vv", name="ps_a")
            for e in range(ec):
                nc.tensor.matmul(
                    ps, w_sb[:, e, :], xts[e],
                    start=(e == 0), stop=(e == ec - 1),
                )
            if dest is None:
                vt = xtp.tile([128, 512], F32R, name="vt", tag="vt", bufs=2)
                nc.scalar.activation(vt, ps, AF.Identity, bias=b_sb)
                vts[sb] = vt
            else:
                nc.scalar.activation(dest[:, tsl], ps, AF.Identity, bias=b_sb)
        if sb > 0:
            emit_transposes(sb - 1, vts.pop(sb - 1))
        if sb == nsb - 1:
            emit_transposes(sb, vts.pop(sb))

    def emit_epilogue(sb):
        tsl = bass.ts(sb, 512)
        rowsum = rowsums.pop(sb)
        nc.sync.dma_start(out=rs_scratch[:, tsl], in_=rowsum[0:1, :, :])
        rinv = outp.tile([128, 512], F32, name="rinv", tag="rinv", bufs=2)
        for h in range(HPC):
            nc.sync.dma_start(
                out=rinv[h * D : (h + 1) * D, :],
                in_=bass.AP(
                    tensor=rs_scratch.tensor,
                    offset=rs_scratch.offset + h * s + sb * 512,
                    ap=[[0, D], [1, 512]],
                ),
            )
        nc.vector.reciprocal(rinv, rinv)
        nc.vector.tensor_mul(ctxT_sb[:, tsl], ctxT_sb[:, tsl], rinv)
        for c in range(4 * sb, 4 * sb + 4):
            for half in range(2):
                out_ps = psum.tile([128, 512], F32, tag="vv", name="out_ps")
                nc.tensor.matmul(
                    out_ps,
                    ctxT_sb[:, c * 128 : (c + 1) * 128],
                    wout_sb[:, half * 512 : (half + 1) * 512],
                    start=True, stop=True,
                )
                out_sb = outp.tile([128, 512], F32, name="out_sb")
                nc.vector.tensor_add(
                    out_sb, out_ps, bout_rep[:, half * 512 : (half + 1) * 512]
                )
                nc.sync.dma_start(
                    out=out[
                        c * 128 : (c + 1) * 128, half * 512 : (half + 1) * 512
                    ],
                    in_=out_sb,
                )

    emit_a(0)
    if nsb > 1:
        emit_a(1)
    for sb in range(nsb):
        tsl = bass.ts(sb, 512)
        ctx_ps = {}
        for h in range(HPC):
            ctx_ps[h] = psum.tile(
                [128, 512], F32, tag=f"ctx{h}", bufs=1, name="ctx_ps"
            )
        npair = 2 * sb + 2
        for m in range(npair):
            stp = {}
            ats = {}
            for h in range(HPC):
                stp[h] = psum.tile(
                    [128, 2, 512], F32, tag=f"st{h}", bufs=1, name="stp"
                )
            for p in range(2):
                for h in range(HPC):
                    hsl = slice(h * D, (h + 1) * D)
                    nc.tensor.matmul(
                        stp[h][:, p, :],
                        kt_sb[hsl, (2 * m + p) * 128 : (2 * m + p + 1) * 128],
                        qt_sb[hsl, tsl],
                        start=True, stop=True,
                    )
            for h in range(HPC):
                at = atp.tile([128, 2, 512], ATT_DT, name="at")
                ats[h] = at
                nc.scalar.activation(at, stp[h], AF.Exp, scale=0.125)
                for p in range(2):
                    r = 2 * m + p - 4 * sb
                    if r >= 0:
                        nc.vector.tensor_mul(
                            at[:, p, :], at[:, p, :], masks_sb[:, r, :]
                        )
            for h in range(HPC):
                for p in range(2):
                    j = 2 * m + p
                    nc.tensor.matmul(
                        ctx_ps[h][0 : D + 1, :],
                        v_sb[:, j, h, :],
                        ats[h][:, p, :],
                        start=(j == 0), stop=(j == 2 * npair - 1),
                    )
            if m == 1 and sb + 2 < nsb:
                emit_a(sb + 2)
            if sb > 0 and m == min(2, npair - 1):
                emit_epilogue(sb - 1)
        rowsum = outp.tile([1, HPC, 512], F32, name="rowsum", tag="rs", bufs=2)
        rowsums[sb] = rowsum
        for h in range(HPC):
            hsl = slice(h * D, (h + 1) * D)
            nc.scalar.copy(ctxT_sb[hsl, tsl], ctx_ps[h][0:D, :])
            nc.vector.tensor_copy(rowsum[0:1, h, :], ctx_ps[h][D : D + 1, :])
    emit_epilogue(nsb - 1)


def build(s=S):
    nc = bacc.Bacc(
        "TRN2",
        target_bir_lowering=False,
        debug=False,
        num_devices=NCORES,
    )
    xT = nc.dram_tensor("xT", [E, s], F32R, kind="ExternalInput")
    wq = nc.dram_tensor("wq", [E, HD], F32R, kind="ExternalInput")
    wk = nc.dram_tensor("wk", [E, HD], F32R, kind="ExternalInput")
    wv = nc.dram_tensor("wv", [E, HD], F32R, kind="ExternalInput")
    bq = nc.dram_tensor("bq", [HD], F32, kind="ExternalInput")
    bk = nc.dram_tensor("bk", [HD], F32, kind="ExternalInput")
    bv = nc.dram_tensor("bv", [HD], F32, kind="ExternalInput")
    wout = nc.dram_tensor("wout", [HD, E], F32R, kind="ExternalInput")
    bout8 = nc.dram_tensor("bout8", [E], F32, kind="ExternalInput")
    masks = nc.dram_tensor("masks", [128, 4 * 512], F32, kind="ExternalInput")
    rs_scratch = nc.dram_tensor("rs_scratch", [HPC, s], F32)
    out = nc.dram_tensor("out", [s, E], F32, kind="ExternalOutput")

    from contextlib import ExitStack

    with tile.TileContext(nc) as tc, ExitStack() as ctx:
        emit(
            ctx, tc,
            xT[:], wq[:], wk[:], wv[:], bq[:], bk[:], bv[:],
            wout[:], bout8[:], masks[:], rs_scratch[:], out[:],
            s=s,
        )
    nc.compile()
    return nc


def make_masks():
    sl, tl = np.mgrid[0:128, 0:512]
    m = np.zeros((128, 4, 512), np.float32)
    for r in range(4):
        m[:, r, :] = (tl - 128 * r - sl >= 0).astype(np.float32)
    return np.ascontiguousarray(m.reshape(128, 4 * 512))


def make_in_maps(x, Wqkv, bqkv, Wout, bout, s=S):
    xT = np.ascontiguousarray(np.asarray(x, np.float32)[0].T)
    Wqkv = np.asarray(Wqkv, np.float32)
    bqkv = np.asarray(bqkv, np.float32)
    Wout = np.asarray(Wout, np.float32)
    bout = np.asarray(bout, np.float32)
    masks = make_masks()
    bout8 = np.ascontiguousarray(bout / NCORES)

    in_maps = []
    for m in range(NCORES):
        heads = range(HPC * m, HPC * (m + 1))
        cols = np.concatenate(
            [np.arange(h * D, (h + 1) * D) for h in heads]
        )
        in_maps.append(
            {
                "xT": xT,
                "wq": np.ascontiguousarray(Wqkv[:, 0 * E + cols]),
                "wk": np.ascontiguousarray(Wqkv[:, 1 * E + cols]),
                "wv": np.ascontiguousarray(Wqkv[:, 2 * E + cols]),
                "bq": np.ascontiguousarray(bqkv[0 * E + cols]),
                "bk": np.ascontiguousarray(bqkv[1 * E + cols]),
                "bv": np.ascontiguousarray(bqkv[2 * E + cols]),
                "wout": np.ascontiguousarray(Wout[HD * m : HD * (m + 1), :]),
                "bout8": bout8,
                "masks": masks,
            }
        )
    return in_maps


_NC_CACHE = {}


def _get_nc(s=S):
    if s not in _NC_CACHE:
        _NC_CACHE[s] = build(s)
    return _NC_CACHE[s]


def kernel(x, Wqkv, bqkv, Wout, bout):
    nc = _get_nc()
    in_maps = make_in_maps(x, Wqkv, bqkv, Wout, bout)
    res = run_bass_kernel_spmd(nc, in_maps, list(range(NCORES)))
    acc = np.zeros((S, E), np.float64)
    for r in res.results:
        acc += r["out"]
    return acc.astype(np.float32).reshape(1, S, E)


# revision 32
# speedup vs baseline: 1.4190x; 1.1143x over previous
"""Bass/Trainium2 kernel for causal MHA (B=1, S=4096, E=1024, H=16, D=64).

Sharding: tensor-parallel over heads across 8 NeuronCores (2 heads/core).
Each core computes q/k/v projections for its 2 heads, causal softmax
attention, and a partial out-projection (contraction over its 128 context
dims). Partials are summed on the host; the output bias is added on-device
as bout/8 per core so the host sum reproduces a single bias add.

Per-core device algorithm (head-packed layout, hd = 2 heads x 64 dims):
  A) QT/KT [128(hd), S] and V [S, hd] from xT tiles (full 128x128 matmuls)
  B) per t-superblock of 512, per head: S^T = K^T.T @ Q^T in [s, t] layout,
     exp via ACT (scale=1/8 fused), causal mask on diagonal-crossing blocks
     via a multiplicative mask, then ctx' += V'.T @ A^T where V' has an
     appended ones column, so ctx'[64] accumulates the softmax denominator.
  C) reciprocal + partition-broadcast of denominators, normalize ctxT,
     out_partial = ctxT.T @ Wout (+ bout/8), DMA out.

All matmuls use float32r (full-rate fp32 PE mode at moving-dim 512).
"""

import sys

import numpy as np

if "/opt/trn_rl_repo" not in sys.path:
    sys.path.append("/opt/trn_rl_repo")

import concourse.bass as bass
import concourse.mybir as mybir
import concourse.tile as tile
from concourse import bacc
from concourse.bass_utils import run_bass_kernel_spmd
from concourse.masks import make_identity

F32 = mybir.dt.float32
F32R = mybir.dt.float32r
BF16 = mybir.dt.bfloat16
AF = mybir.ActivationFunctionType

# bf16 operands for the attention matmuls (q/k/A/V): PE runs bf16 at
# 1 cycle/row vs fp32r's 2, and attention dominates PE time. qkv- and
# out-projections stay f32r; accumulation is always fp32 in PSUM.
ATT_BF16 = True
ATT_DT = BF16 if ATT_BF16 else F32R
# bf16 for the projection matmuls too (x tiles, Wq/k/v, Wout, ctxT):
# halves remaining PE row-cycles and the 16MB xT DMA.
PROJ_BF16 = True
PROJ_DT = BF16 if PROJ_BF16 else F32R

S = 4096
E = 1024
H = 16
D = 64
NCORES = 8
HPC = H // NCORES  # heads per core
HD = HPC * D  # packed head dims per core (=128)

def emit(ctx, tc, xT, wq, wk, wv, bq, bk, bv, wout, bout8, masks, rs_scratch, out, s=S):
    """Emit the per-core program. s = sequence length (parameterized so the
    simulator can run a scaled-down version)."""
    nc = tc.nc
    ec = E // 128  # contraction chunks for qkv proj
    nsb = s // 512  # t-superblocks
    nsc = s // 128  # s-chunks

    const = ctx.enter_context(tc.tile_pool(name="const", bufs=1))
    big = ctx.enter_context(tc.tile_pool(name="big", bufs=1))
    xtp = ctx.enter_context(tc.tile_pool(name="xtp", bufs=12))
    atp = ctx.enter_context(tc.tile_pool(name="atp", bufs=6))
    outp = ctx.enter_context(tc.tile_pool(name="outp", bufs=3))
    psum = ctx.enter_context(tc.tile_pool(name="psum", bufs=2, space="PSUM"))

    # ---- constants / weights in SBUF ----
    wq_sb = const.tile([128, ec, HD], PROJ_DT)  # [e_in_chunk, e_chunk, hd]
    wk_sb = const.tile([128, ec, HD], PROJ_DT)
    wv_sb = const.tile([128, ec, HD], PROJ_DT)
    for w_sb, w_dram in ((wq_sb, wq), (wk_sb, wk), (wv_sb, wv)):
        nc.sync.dma_start(
            out=w_sb, in_=w_dram.rearrange("(c p) n -> p c n", p=128)
        )
    bq_sb = const.tile([HD, 1], F32)
    bk_sb = const.tile([HD, 1], F32)
    nc.sync.dma_start(
        out=bq_sb,
        in_=bass.AP(tensor=bq.tensor, offset=bq.offset, ap=[[1, HD], [1, 1]]),
    )
    nc.sync.dma_start(
        out=bk_sb,
        in_=bass.AP(tensor=bk.tensor, offset=bk.offset, ap=[[1, HD], [1, 1]]),
    )
    bv_sb = const.tile([HD, 1], F32)
    nc.sync.dma_start(
        out=bv_sb,
        in_=bass.AP(tensor=bv.tensor, offset=bv.offset, ap=[[1, HD], [1, 1]]),
    )
    wout_sb = const.tile([HD, E], PROJ_DT)
    nc.sync.dma_start(out=wout_sb, in_=wout)
    bout_rep = const.tile([128, E], F32)
    nc.sync.dma_start(
        out=bout_rep,
        in_=bass.AP(
            tensor=bout8.tensor, offset=bout8.offset, ap=[[0, 128], [1, E]]
        ),
    )
    masks_sb = const.tile([128, 4, 512], F32)
    nc.sync.dma_start(out=masks_sb, in_=masks.rearrange("p (r n) -> p r n", r=4))
    ident32 = const.tile([128, 128], F32)
    make_identity(nc, ident32)
    ident = const.tile([128, 128], PROJ_DT)
    nc.vector.tensor_copy(ident, ident32)
    ones32 = const.tile([128, 1], F32)
    nc.vector.memset(ones32, 1.0)

    # ---- persistent activations ----
    qt_sb = big.tile([128, s], ATT_DT)  # [hd, t]
    kt_sb = big.tile([128, s], ATT_DT)  # [hd, s]
    v_sb = big.tile([128, nsc, HPC, D + 1], ATT_DT)  # [s_in_chunk, chunk, h, d']
    ctxT_sb = big.tile([128, s], PROJ_DT)  # [hd, t]

    # V' ones column (free-dim-broadcast copy so the f32r write is rounded)
    nc.vector.tensor_copy(
        v_sb[:, :, :, D : D + 1],
        bass.AP(
            tensor=ones32.tensor,
            offset=ones32.offset,
            ap=[list(ones32.ap[0]), [0, nsc], [0, HPC], [0, 1]],
        ),
    )

    # ---- phases A+B interleaved ----
    # Phase A superblocks (qkv projection) are emitted two superblocks ahead
    # of phase B (attention), injected into B's pair loop, so the PE always
    # has projection matmuls to run while ACT works through exp calls.
    # A accumulates q/k/v sequentially into single "vv"-tag psum slots so the
    # st0/st1 tags stay free for B's score pairs.
    rowsums = {}

    def emit_transposes(sb, vt):
        for r in range(4):
            j = sb * 4 + r
            ps_t = psum.tile([128, 128], PROJ_DT, tag="vv", name="ps_t")
            nc.tensor.transpose(ps_t, vt[:, r * 128 : (r + 1) * 128], ident)
            for h in range(HPC):
                nc.vector.tensor_copy(
                    v_sb[:, j, h, 0:D], ps_t[:, h * D : (h + 1) * D]
                )

    vts = {}

    def emit_a(sb):
        tsl = bass.ts(sb, 512)
        xts = []
        for e in range(ec):
            xt = xtp.tile([128, 512], PROJ_DT, name="xt")
            nc.sync.dma_start(out=xt, in_=xT[e * 128 : (e + 1) * 128, tsl])
            xts.append(xt)
        for w_sb, b_sb, dest in (
            (wq_sb, bq_sb, qt_sb), (wk_sb, bk_sb, kt_sb), (wv_sb, bv_sb, None)
        ):
            ps = psum.tile([128, 512], F32, tag="vv", name="ps_a")
            for e in range(ec):
                nc.tensor.matmul(
                    ps, w_sb[:, e, :], xts[e],
                    start=(e == 0), stop=(e == ec - 1),
                )
            if dest is None:
                vt = xtp.tile([128, 512], PROJ_DT, name="vt", tag="vt", bufs=2)
                nc.scalar.activation(vt, ps, AF.Identity, bias=b_sb)
                vts[sb] = vt
            else:
                nc.scalar.activation(dest[:, tsl], ps, AF.Identity, bias=b_sb)
        if sb > 0:
            emit_transposes(sb - 1, vts.pop(sb - 1))
        if sb == nsb - 1:
            emit_transposes(sb, vts.pop(sb))

    def emit_epilogue(sb):
        tsl = bass.ts(sb, 512)
        rowsum = rowsums.pop(sb)
        nc.sync.dma_start(out=rs_scratch[:, tsl], in_=rowsum[0:1, :, :])
        rinv = outp.tile([128, 512], F32, name="rinv", tag="rinv", bufs=2)
        for h in range(HPC):
            nc.sync.dma_start(
                out=rinv[h * D : (h + 1) * D, :],
                in_=bass.AP(
                    tensor=rs_scratch.tensor,
                    offset=rs_scratch.offset + h * s + sb * 512,
                    ap=[[0, D], [1, 512]],
                ),
            )
        nc.vector.reciprocal(rinv, rinv)
        nc.vector.tensor_mul(ctxT_sb[:, tsl], ctxT_sb[:, tsl], rinv)
        for c in range(4 * sb, 4 * sb + 4):
            for half in range(2):
                out_ps = psum.tile([128, 512], F32, tag="vv", name="out_ps")
                nc.tensor.matmul(
                    out_ps,
                    ctxT_sb[:, c * 128 : (c + 1) * 128],
                    wout_sb[:, half * 512 : (half + 1) * 512],
                    start=True, stop=True,
                )
                out_sb = outp.tile([128, 512], F32, name="out_sb")
                nc.vector.tensor_add(
                    out_sb, out_ps, bout_rep[:, half * 512 : (half + 1) * 512]
                )
                nc.sync.dma_start(
                    out=out[
                        c * 128 : (c + 1) * 128, half * 512 : (half + 1) * 512
                    ],
                    in_=out_sb,
                )

    emit_a(0)
    if nsb > 1:
        emit_a(1)
    for sb in range(nsb):
        tsl = bass.ts(sb, 512)
        ctx_ps = {}
        for h in range(HPC):
            ctx_ps[h] = psum.tile(
                [128, 512], F32, tag=f"ctx{h}", bufs=1, name="ctx_ps"
            )
        npair = 2 * sb + 2
        for m in range(npair):
            stp = {}
            ats = {}
            for h in range(HPC):
                stp[h] = psum.tile(
                    [128, 2, 512], F32, tag=f"st{h}", bufs=1, name="stp"
                )
            for p in range(2):
                for h in range(HPC):
                    hsl = slice(h * D, (h + 1) * D)
                    nc.tensor.matmul(
                        stp[h][:, p, :],
                        kt_sb[hsl, (2 * m + p) * 128 : (2 * m + p + 1) * 128],
                        qt_sb[hsl, tsl],
                        start=True, stop=True,
                    )
            for h in range(HPC):
                at = atp.tile([128, 2, 512], ATT_DT, name="at")
                ats[h] = at
                nc.scalar.activation(at, stp[h], AF.Exp, scale=0.125)
                for p in range(2):
                    r = 2 * m + p - 4 * sb
                    if r >= 0:
                        nc.vector.tensor_mul(
                            at[:, p, :], at[:, p, :], masks_sb[:, r, :]
                        )
            for h in range(HPC):
                for p in range(2):
                    j = 2 * m + p
                    nc.tensor.matmul(
                        ctx_ps[h][0 : D + 1, :],
                        v_sb[:, j, h, :],
                        ats[h][:, p, :],
                        start=(j == 0), stop=(j == 2 * npair - 1),
                    )
            if m == 1 and sb + 2 < nsb:
                emit_a(sb + 2)
            if sb > 0 and m == min(2, npair - 1):
                emit_epilogue(sb - 1)
        rowsum = outp.tile([1, HPC, 512], F32, name="rowsum", tag="rs", bufs=2)
        rowsums[sb] = rowsum
        for h in range(HPC):
            hsl = slice(h * D, (h + 1) * D)
            nc.scalar.copy(ctxT_sb[hsl, tsl], ctx_ps[h][0:D, :])
            nc.vector.tensor_copy(rowsum[0:1, h, :], ctx_ps[h][D : D + 1, :])
    emit_epilogue(nsb - 1)


def build(s=S):
    nc = bacc.Bacc(
        "TRN2",
        target_bir_lowering=False,
        debug=False,
        num_devices=NCORES,
    )
    xT = nc.dram_tensor("xT", [E, s], PROJ_DT, kind="ExternalInput")
    wq = nc.dram_tensor("wq", [E, HD], PROJ_DT, kind="ExternalInput")
    wk = nc.dram_tensor("wk", [E, HD], PROJ_DT, kind="ExternalInput")
    wv = nc.dram_tensor("wv", [E, HD], PROJ_DT, kind="ExternalInput")
    bq = nc.dram_tensor("bq", [HD], F32, kind="ExternalInput")
    bk = nc.dram_tensor("bk", [HD], F32, kind="ExternalInput")
    bv = nc.dram_tensor("bv", [HD], F32, kind="ExternalInput")
    wout = nc.dram_tensor("wout", [HD, E], PROJ_DT, kind="ExternalInput")
    bout8 = nc.dram_tensor("bout8", [E], F32, kind="ExternalInput")
    masks = nc.dram_tensor("masks", [128, 4 * 512], F32, kind="ExternalInput")
    rs_scratch = nc.dram_tensor("rs_scratch", [HPC, s], F32)
    out = nc.dram_tensor("out", [s, E], F32, kind="ExternalOutput")

    from contextlib import ExitStack

    with tile.TileContext(nc) as tc, ExitStack() as ctx:
        emit(
            ctx, tc,
            xT[:], wq[:], wk[:], wv[:], bq[:], bk[:], bv[:],
            wout[:], bout8[:], masks[:], rs_scratch[:], out[:],
            s=s,
        )
    nc.compile()
    return nc


def make_masks():
    sl, tl = np.mgrid[0:128, 0:512]
    m = np.zeros((128, 4, 512), np.float32)
    for r in range(4):
        m[:, r, :] = (tl - 128 * r - sl >= 0).astype(np.float32)
    return np.ascontiguousarray(m.reshape(128, 4 * 512))


def make_in_maps(x, Wqkv, bqkv, Wout, bout, s=S):
    proj_np = mybir.dt.np(PROJ_DT)
    xT = np.ascontiguousarray(np.asarray(x, np.float32)[0].T).astype(proj_np)
    Wqkv = np.asarray(Wqkv, np.float32)
    bqkv = np.asarray(bqkv, np.float32)
    Wout = np.asarray(Wout, np.float32)
    bout = np.asarray(bout, np.float32)
    masks = make_masks()
    bout8 = np.ascontiguousarray(bout / NCORES)

    in_maps = []
    for m in range(NCORES):
        heads = range(HPC * m, HPC * (m + 1))
        cols = np.concatenate(
            [np.arange(h * D, (h + 1) * D) for h in heads]
        )
        in_maps.append(
            {
                "xT": xT,
                "wq": np.ascontiguousarray(Wqkv[:, 0 * E + cols]).astype(proj_np),
                "wk": np.ascontiguousarray(Wqkv[:, 1 * E + cols]).astype(proj_np),
                "wv": np.ascontiguousarray(Wqkv[:, 2 * E + cols]).astype(proj_np),
                "bq": np.ascontiguousarray(bqkv[0 * E + cols]),
                "bk": np.ascontiguousarray(bqkv[1 * E + cols]),
                "bv": np.ascontiguousarray(bqkv[2 * E + cols]),
                "wout": np.ascontiguousarray(Wout[HD * m : HD * (m + 1), :]).astype(proj_np),
                "bout8": bout8,
                "masks": masks,
            }
        )
    return in_maps


_NC_CACHE = {}


def _get_nc(s=S):
    if s not in _NC_CACHE:
        _NC_CACHE[s] = build(s)
    return _NC_CACHE[s]


def kernel(x, Wqkv, bqkv, Wout, bout):
    nc = _get_nc()
    in_maps = make_in_maps(x, Wqkv, bqkv, Wout, bout)
    res = run_bass_kernel_spmd(nc, in_maps, list(range(NCORES)))
    acc = np.zeros((S, E), np.float64)
    for r in res.results:
        acc += r["out"]
    return acc.astype(np.float32).reshape(1, S, E)


# revision 35
# speedup vs baseline: 1.7361x; 1.2235x over previous
"""Bass/Trainium2 kernel for causal MHA (B=1, S=4096, E=1024, H=16, D=64).

Sharding: tensor-parallel over heads across 8 NeuronCores (2 heads/core).
Each core computes q/k/v projections for its 2 heads, causal softmax
attention, and a partial out-projection (contraction over its 128 context
dims). Partials are summed on the host; the output bias is added on-device
as bout/8 per core so the host sum reproduces a single bias add.

Per-core device algorithm (head-packed layout, hd = 2 heads x 64 dims):
  A) QT/KT [128(hd), S] and V [S, hd] from xT tiles (full 128x128 matmuls)
  B) per t-superblock of 512, per head: S^T = K^T.T @ Q^T in [s, t] layout,
     exp via ACT (scale=1/8 fused), causal mask on diagonal-crossing blocks
     via a multiplicative mask, then ctx' += V'.T @ A^T where V' has an
     appended ones column, so ctx'[64] accumulates the softmax denominator.
  C) reciprocal + partition-broadcast of denominators, normalize ctxT,
     out_partial = ctxT.T @ Wout (+ bout/8), DMA out.

All matmuls use float32r (full-rate fp32 PE mode at moving-dim 512).
"""

import sys

import numpy as np

if "/opt/trn_rl_repo" not in sys.path:
    sys.path.append("/opt/trn_rl_repo")

import concourse.bass as bass
import concourse.mybir as mybir
import concourse.tile as tile
from concourse import bacc
from concourse.bass_utils import run_bass_kernel_spmd
from concourse.masks import make_identity

F32 = mybir.dt.float32
F32R = mybir.dt.float32r
BF16 = mybir.dt.bfloat16
AF = mybir.ActivationFunctionType

# bf16 operands for the attention matmuls (q/k/A/V): PE runs bf16 at
# 1 cycle/row vs fp32r's 2, and attention dominates PE time. qkv- and
# out-projections stay f32r; accumulation is always fp32 in PSUM.
ATT_BF16 = True
ATT_DT = BF16 if ATT_BF16 else F32R
# bf16 for the projection matmuls too (x tiles, Wq/k/v, Wout, ctxT):
# halves remaining PE row-cycles and the 16MB xT DMA.
PROJ_BF16 = True
PROJ_DT = BF16 if PROJ_BF16 else F32R

S = 4096
E = 1024
H = 16
D = 64
NCORES = 8
HPC = H // NCORES  # heads per core
HD = HPC * D  # packed head dims per core (=128)

def emit(ctx, tc, xT, wq, wk, wv, bq, bk, bv, wout, bout8, rs_scratch, out, s=S):
    """Emit the per-core program. s = sequence length (parameterized so the
    simulator can run a scaled-down version)."""
    nc = tc.nc
    ec = E // 128  # contraction chunks for qkv proj
    nsb = s // 512  # t-superblocks
    nsc = s // 128  # s-chunks

    const = ctx.enter_context(tc.tile_pool(name="const", bufs=1))
    big = ctx.enter_context(tc.tile_pool(name="big", bufs=1))
    xtp = ctx.enter_context(tc.tile_pool(name="xtp", bufs=12))
    atp = ctx.enter_context(tc.tile_pool(name="atp", bufs=6))
    outp = ctx.enter_context(tc.tile_pool(name="outp", bufs=3))
    psum = ctx.enter_context(tc.tile_pool(name="psum", bufs=2, space="PSUM"))

    # ---- constants / weights in SBUF ----
    wq_sb = const.tile([128, ec, HD], PROJ_DT)  # [e_in_chunk, e_chunk, hd]
    wk_sb = const.tile([128, ec, HD], PROJ_DT)
    wv_sb = const.tile([128, ec, HD], PROJ_DT)
    for w_sb, w_dram in ((wq_sb, wq), (wk_sb, wk), (wv_sb, wv)):
        nc.sync.dma_start(
            out=w_sb, in_=w_dram.rearrange("(c p) n -> p c n", p=128)
        )
    bq_sb = const.tile([HD, 1], F32)
    bk_sb = const.tile([HD, 1], F32)
    nc.sync.dma_start(
        out=bq_sb,
        in_=bass.AP(tensor=bq.tensor, offset=bq.offset, ap=[[1, HD], [1, 1]]),
    )
    nc.sync.dma_start(
        out=bk_sb,
        in_=bass.AP(tensor=bk.tensor, offset=bk.offset, ap=[[1, HD], [1, 1]]),
    )
    bv_sb = const.tile([HD, 1], F32)
    nc.sync.dma_start(
        out=bv_sb,
        in_=bass.AP(tensor=bv.tensor, offset=bv.offset, ap=[[1, HD], [1, 1]]),
    )
    wout_sb = const.tile([HD, E], PROJ_DT)
    nc.sync.dma_start(out=wout_sb, in_=wout)
    bout_rep = const.tile([128, E], F32)
    nc.sync.dma_start(
        out=bout_rep,
        in_=bass.AP(
            tensor=bout8.tensor, offset=bout8.offset, ap=[[0, 128], [1, E]]
        ),
    )
    ident32 = const.tile([128, 128], F32)
    make_identity(nc, ident32)
    ident = const.tile([128, 128], PROJ_DT)
    nc.vector.tensor_copy(ident, ident32)
    ones32 = const.tile([128, 1], F32)
    nc.vector.memset(ones32, 1.0)

    # ---- persistent activations ----
    qt_sb = big.tile([128, s], ATT_DT)  # [hd, t]
    kt_sb = big.tile([128, s], ATT_DT)  # [hd, s]
    v_sb = big.tile([128, nsc, HPC, D + 1], ATT_DT)  # [s_in_chunk, chunk, h, d']
    ctxT_sb = big.tile([128, s], PROJ_DT)  # [hd, t]

    # V' ones column (free-dim-broadcast copy so the f32r write is rounded)
    nc.vector.tensor_copy(
        v_sb[:, :, :, D : D + 1],
        bass.AP(
            tensor=ones32.tensor,
            offset=ones32.offset,
            ap=[list(ones32.ap[0]), [0, nsc], [0, HPC], [0, 1]],
        ),
    )

    # ---- phases A+B interleaved ----
    # Phase A superblocks (qkv projection) are emitted two superblocks ahead
    # of phase B (attention), injected into B's pair loop, so the PE always
    # has projection matmuls to run while ACT works through exp calls.
    # A accumulates q/k/v sequentially into single "vv"-tag psum slots so the
    # st0/st1 tags stay free for B's score pairs.
    rowsums = {}

    def emit_transposes(sb, vt):
        for r in range(4):
            j = sb * 4 + r
            ps_t = psum.tile([128, 128], PROJ_DT, tag="vv", name="ps_t")
            nc.tensor.transpose(ps_t, vt[:, r * 128 : (r + 1) * 128], ident)
            for h in range(HPC):
                nc.vector.tensor_copy(
                    v_sb[:, j, h, 0:D], ps_t[:, h * D : (h + 1) * D]
                )

    vts = {}

    def emit_a(sb):
        tsl = bass.ts(sb, 512)
        xts = []
        for e in range(ec):
            xt = xtp.tile([128, 512], PROJ_DT, name="xt")
            nc.sync.dma_start(out=xt, in_=xT[e * 128 : (e + 1) * 128, tsl])
            xts.append(xt)
        for w_sb, b_sb, dest in (
            (wq_sb, bq_sb, qt_sb), (wk_sb, bk_sb, kt_sb), (wv_sb, bv_sb, None)
        ):
            ps = psum.tile([128, 512], F32, tag="vv", name="ps_a")
            for e in range(ec):
                nc.tensor.matmul(
                    ps, w_sb[:, e, :], xts[e],
                    start=(e == 0), stop=(e == ec - 1),
                )
            if dest is None:
                vt = xtp.tile([128, 512], PROJ_DT, name="vt", tag="vt", bufs=2)
                nc.scalar.activation(vt, ps, AF.Identity, bias=b_sb)
                vts[sb] = vt
            else:
                nc.vector.tensor_scalar_add(dest[:, tsl], ps, b_sb)
        if sb > 0:
            emit_transposes(sb - 1, vts.pop(sb - 1))
        if sb == nsb - 1:
            emit_transposes(sb, vts.pop(sb))

    def emit_epilogue(sb, nsplit=1):
        tsl = bass.ts(sb, 512)
        rowsum = rowsums.pop(sb)
        nc.sync.dma_start(out=rs_scratch[:, tsl], in_=rowsum[0:1, :, :])
        rinv = outp.tile([128, 512], F32, name="rinv", tag="rinv", bufs=2)
        for h in range(HPC):
            nc.sync.dma_start(
                out=rinv[h * D : (h + 1) * D, :],
                in_=bass.AP(
                    tensor=rs_scratch.tensor,
                    offset=rs_scratch.offset + h * s + sb * 512,
                    ap=[[0, D], [1, 512]],
                ),
            )
        w = 512 // nsplit
        for part in range(nsplit):
            psl = slice(part * w, (part + 1) * w)
            nc.vector.reciprocal_approx_fast(out=rinv[:, psl], in_=rinv[:, psl])
            nc.vector.tensor_mul(
                ctxT_sb[:, sb * 512 + part * w : sb * 512 + (part + 1) * w],
                ctxT_sb[:, sb * 512 + part * w : sb * 512 + (part + 1) * w],
                rinv[:, psl],
            )
            c0 = 4 * sb + part * (4 // nsplit)
            for c in range(c0, c0 + 4 // nsplit):
                for half in range(2):
                    out_ps = psum.tile([128, 512], F32, tag="vv", name="out_ps")
                    nc.tensor.matmul(
                        out_ps,
                        ctxT_sb[:, c * 128 : (c + 1) * 128],
                        wout_sb[:, half * 512 : (half + 1) * 512],
                        start=True, stop=True,
                    )
                    out_sb = outp.tile([128, 512], F32, name="out_sb")
                    nc.vector.tensor_add(
                        out_sb, out_ps, bout_rep[:, half * 512 : (half + 1) * 512]
                    )
                    nc.sync.dma_start(
                        out=out[
                            c * 128 : (c + 1) * 128, half * 512 : (half + 1) * 512
                        ],
                        in_=out_sb,
                    )

    emit_a(0)
    if nsb > 1:
        emit_a(1)
    for sb in range(nsb):
        tsl = bass.ts(sb, 512)
        ctx_ps = {}
        for h in range(HPC):
            ctx_ps[h] = psum.tile(
                [128, 512], F32, tag=f"ctx{h}", bufs=1, name="ctx_ps"
            )
        npair = 2 * sb + 2
        for m in range(npair):
            stp = {}
            ats = {}
            for h in range(HPC):
                stp[h] = psum.tile(
                    [128, 2, 512], F32, tag=f"st{h}", bufs=1, name="stp"
                )
            for p in range(2):
                for h in range(HPC):
                    hsl = slice(h * D, (h + 1) * D)
                    nc.tensor.matmul(
                        stp[h][:, p, :],
                        kt_sb[hsl, (2 * m + p) * 128 : (2 * m + p + 1) * 128],
                        qt_sb[hsl, tsl],
                        start=True, stop=True,
                    )
            for h in range(HPC):
                at = atp.tile([128, 2, 512], ATT_DT, name="at")
                ats[h] = at
                nc.scalar.activation(at, stp[h], AF.Exp, scale=0.125)
                for p in range(2):
                    r = 2 * m + p - 4 * sb
                    if r >= 0:
                        # causal mask: keep t_local >= 128*r + s_local
                        nc.gpsimd.affine_select(
                            out=at[:, p, :],
                            in_=at[:, p, :],
                            compare_op=mybir.AluOpType.is_ge,
                            fill=0.0,
                            base=-128 * r,
                            pattern=[[1, 512]],
                            channel_multiplier=-1,
                        )
            for h in range(HPC):
                for p in range(2):
                    j = 2 * m + p
                    nc.tensor.matmul(
                        ctx_ps[h][0 : D + 1, :],
                        v_sb[:, j, h, :],
                        ats[h][:, p, :],
                        start=(j == 0), stop=(j == 2 * npair - 1),
                    )
            if m == 1 and sb + 2 < nsb:
                emit_a(sb + 2)
            if sb > 0 and m == min(2, npair - 1):
                emit_epilogue(sb - 1)
        rowsum = outp.tile([1, HPC, 512], F32, name="rowsum", tag="rs", bufs=2)
        rowsums[sb] = rowsum
        for h in range(HPC):
            hsl = slice(h * D, (h + 1) * D)
            if h == 0:
                nc.scalar.copy(ctxT_sb[hsl, tsl], ctx_ps[h][0:D, :])
            else:
                nc.vector.tensor_copy(ctxT_sb[hsl, tsl], ctx_ps[h][0:D, :])
            nc.vector.tensor_copy(rowsum[0:1, h, :], ctx_ps[h][D : D + 1, :])
    emit_epilogue(nsb - 1, nsplit=4)


def build(s=S):
    nc = bacc.Bacc(
        "TRN2",
        target_bir_lowering=False,
        debug=False,
        num_devices=NCORES,
    )
    xT = nc.dram_tensor("xT", [E, s], PROJ_DT, kind="ExternalInput")
    wq = nc.dram_tensor("wq", [E, HD], PROJ_DT, kind="ExternalInput")
    wk = nc.dram_tensor("wk", [E, HD], PROJ_DT, kind="ExternalInput")
    wv = nc.dram_tensor("wv", [E, HD], PROJ_DT, kind="ExternalInput")
    bq = nc.dram_tensor("bq", [HD], F32, kind="ExternalInput")
    bk = nc.dram_tensor("bk", [HD], F32, kind="ExternalInput")
    bv = nc.dram_tensor("bv", [HD], F32, kind="ExternalInput")
    wout = nc.dram_tensor("wout", [HD, E], PROJ_DT, kind="ExternalInput")
    bout8 = nc.dram_tensor("bout8", [E], F32, kind="ExternalInput")
    rs_scratch = nc.dram_tensor("rs_scratch", [HPC, s], F32)
    out = nc.dram_tensor("out", [s, E], F32, kind="ExternalOutput")

    from contextlib import ExitStack

    with tile.TileContext(nc) as tc, ExitStack() as ctx:
        emit(
            ctx, tc,
            xT[:], wq[:], wk[:], wv[:], bq[:], bk[:], bv[:],
            wout[:], bout8[:], rs_scratch[:], out[:],
            s=s,
        )
    nc.compile()
    return nc


def make_masks():
    sl, tl = np.mgrid[0:128, 0:512]
    m = np.zeros((128, 4, 512), np.float32)
    for r in range(4):
        m[:, r, :] = (tl - 128 * r - sl >= 0).astype(np.float32)
    return np.ascontiguousarray(m.reshape(128, 4 * 512))


def make_in_maps(x, Wqkv, bqkv, Wout, bout, s=S):
    proj_np = mybir.dt.np(PROJ_DT)
    xT = np.ascontiguousarray(np.asarray(x, np.float32)[0].T).astype(proj_np)
    Wqkv = np.asarray(Wqkv, np.float32)
    bqkv = np.asarray(bqkv, np.float32)
    Wout = np.asarray(Wout, np.float32)
    bout = np.asarray(bout, np.float32)
    bout8 = np.ascontiguousarray(bout / NCORES)

    in_maps = []
    for m in range(NCORES):
        heads = range(HPC * m, HPC * (m + 1))
        cols = np.concatenate(
            [np.arange(h * D, (h + 1) * D) for h in heads]
        )
        in_maps.append(
            {
                "xT": xT,
                "wq": np.ascontiguousarray(Wqkv[:, 0 * E + cols]).astype(proj_np),
                "wk": np.ascontiguousarray(Wqkv[:, 1 * E + cols]).astype(proj_np),
                "wv": np.ascontiguousarray(Wqkv[:, 2 * E + cols]).astype(proj_np),
                "bq": np.ascontiguousarray(bqkv[0 * E + cols]),
                "bk": np.ascontiguousarray(bqkv[1 * E + cols]),
                "bv": np.ascontiguousarray(bqkv[2 * E + cols]),
                "wout": np.ascontiguousarray(Wout[HD * m : HD * (m + 1), :]).astype(proj_np),
                "bout8": bout8,
            }
        )
    return in_maps


_NC_CACHE = {}


def _get_nc(s=S):
    if s not in _NC_CACHE:
        _NC_CACHE[s] = build(s)
    return _NC_CACHE[s]


def kernel(x, Wqkv, bqkv, Wout, bout):
    nc = _get_nc()
    in_maps = make_in_maps(x, Wqkv, bqkv, Wout, bout)
    res = run_bass_kernel_spmd(nc, in_maps, list(range(NCORES)))
    acc = np.zeros((S, E), np.float64)
    for r in res.results:
        acc += r["out"]
    return acc.astype(np.float32).reshape(1, S, E)
